# revision 4
# baseline (speedup 1.0000x reference)
"""Multi-head self-attention (RoPE, causal) Trainium2 Bass kernel.

Sharding: 8 cores = 4 batches x 2 head-groups (8 heads each).
Per core, for its batch b and head-group g:
    q/k/v = x_b @ W*[:, g] (+bias), RoPE on q/k, causal softmax attention,
    partial out-projection y @ Wo[g]  -> [2048, 1024] (f32).
Host sums the two head-group partials per batch and adds bo.

Schedule: the whole kernel is emitted as one interleaved weave so all
engines stay fed:
  - stage B (projections+RoPE) is emitted m-group-major with v early;
    attention for a head pair starts as soon as its q/k/v land, so the
    scalar engine's exp (the C-phase pacer) starts ~25% into B
  - steady state pipelines c1(h) [QK+exp+mask] against c2(h-1) [AV],
    with 4 c1 steps per c2 step so PE work spreads across the exp steps
  - stage D (out-projection) is woven with the last head's AV
Engines: scalar does only exp + psum->sbuf copies (one act-table set, no
reloads); RoPE rotate-half runs as SBUF->SBUF DMAs on the sync/gpsimd
queues; RoPE muls + per-qb softmax scale on vector; causal mask on
gpsimd.  kTf is stored paired like qTf and QK uses 64-row lhsT/rhs at
partition offset 0/64 (no zero padding, half the weight-load traffic).
AV keeps v moving (65 cols) over a full-array stationary att block, with
v's ones column making psum col 64 the softmax denominator.  B-scoped
SBUF pools (x, weights, tables) close before stage-D tiles allocate.
"""

import os
import sys

import numpy as np

for _p in ("/opt/trn_rl_repo", "/root/.axon_site/_ro/trn_rl_repo"):
    if os.path.isdir(_p) and _p not in sys.path:
        sys.path.append(_p)

import ml_dtypes  # noqa: E402

BF16 = ml_dtypes.bfloat16

B, S, D_MODEL = 4, 2048, 1024
N_HEADS, HEAD_DIM = 16, 64
N_CORES = 8
HG = 2                      # head groups
HPC = N_HEADS // HG         # heads per core = 8
DL = HPC * HEAD_DIM         # local dims per core = 512
SCALE = HEAD_DIM ** -0.5
P = 128
KC = D_MODEL // P           # k chunks in projections = 8
MB = DL // P                # m blocks (head pairs) = 4
NKB = S // P                # 128-row blocks of sequence = 16
NQT = S // DL               # 512-col q tiles = 4
HH = HEAD_DIM // 2          # 32
QKW = 1024                  # scores psum tile width (2 banks)

# packed causal-trapezoid offsets: att row-block ck covers q in [128*ck, S)
ATT_OFF = [0] * (NKB + 1)
for _ck in range(NKB):
    ATT_OFF[_ck + 1] = ATT_OFF[_ck] + (S - P * _ck)
ATT_TOT = ATT_OFF[NKB]      # 17408

_CACHE = {}


def _build_bass():
    import concourse.tile as tile
    from concourse import bacc, mybir

    dt = mybir.dt
    nc = bacc.Bacc("TRN2", target_bir_lowering=False, debug=False)

    def din(name, shape, d=dt.bfloat16):
        return nc.dram_tensor(name, shape, d, kind="ExternalInput").ap()

    xT_d = din("xT", [D_MODEL, S])
    wq_d = din("wq", [D_MODEL, DL])
    wk_d = din("wk", [D_MODEL, DL])
    wv_d = din("wv", [D_MODEL, DL])
    wo_d = din("wo", [DL, D_MODEL])
    bq_d = din("bqT", [P, MB], dt.float32)
    bk_d = din("bkT", [P, MB], dt.float32)
    bv_d = din("bv", [1, DL])
    cos_d = din("cosT", [P, S])
    sin_d = din("sinT", [P, S])          # sign-folded (pi-basis)
    tri_d = din("tri", [P, P])
    ident_d = din("ident", [P, P])
    o_d = nc.dram_tensor("o", [S, D_MODEL], dt.bfloat16, kind="ExternalOutput").ap()

    FCopy = mybir.ActivationFunctionType.Copy
    FIdent = mybir.ActivationFunctionType.Identity
    FExp = mybir.ActivationFunctionType.Exp
    FLn = mybir.ActivationFunctionType.Ln
    NT = S // DL  # 4 sequence tiles of 512

    with tile.TileContext(nc) as tc:
        with (
            tc.tile_pool(name="persist", bufs=1) as persist,
            tc.tile_pool(name="small", bufs=1) as small,
            tc.tile_pool(name="qpool", bufs=3) as qpool,
            tc.tile_pool(name="kpool", bufs=3) as kpool,
            tc.tile_pool(name="rrow", bufs=2) as rrow,
            tc.tile_pool(name="proj_ps", bufs=2, space="PSUM") as proj_ps,
            tc.tile_pool(name="qk_psA", bufs=1, space="PSUM") as qk_psA,
            tc.tile_pool(name="qk_psB", bufs=1, space="PSUM") as qk_psB,
            tc.tile_pool(name="yT_ps", bufs=2, space="PSUM") as yT_ps_pool,
        ):
            # ---------------- persistent tiles ----------------
            v_sb = persist.tile([P, NKB, HPC, HEAD_DIM + 1], dt.bfloat16,
                                tag="v_sb")
            yT_all = persist.tile([P, MB, S], dt.bfloat16, tag="yT")
            y_mb = {}

            def get_ymb(m):
                if m not in y_mb:
                    y_mb[m] = persist.tile([P, NKB, P], dt.bfloat16,
                                           tag="y_mb", name=f"y_mb{m}",
                                           bufs=3)
                return y_mb[m]
            ident_sb = small.tile([P, P], dt.bfloat16, tag="ident")
            att_tiles = {}

            tri_sb = small.tile([P, P], dt.bfloat16, tag="tri")
            ones_sb = small.tile([1, P], dt.bfloat16, tag="ones")
            b_sbs = {
                "q": small.tile([P, MB], dt.float32, tag="bq", name="bq"),
                "k": small.tile([P, MB], dt.float32, tag="bk", name="bk"),
                "v": small.tile([1, DL], dt.bfloat16, tag="bv", name="bv"),
            }
            nc.vector.memset(ones_sb, 1.0)
            nc.vector.memset(v_sb[:, :, :, HEAD_DIM:HEAD_DIM + 1], 1.0)

            qtiles = {}
            ktiles = {}

            def get_att(h):
                if h not in att_tiles:
                    att_tiles[h] = persist.tile(
                        [P, ATT_TOT], dt.bfloat16, tag=f"att{h % 2}",
                        name=f"att{h}", bufs=1)
                return att_tiles[h]

            # ---------------- stage B emitters ----------------
            def emit_qk_tile(nm, t, m, par):
                """Projection + RoPE for one [128, 512] tile of q or k."""
                w_sb, b_sb = w_sbs[nm], b_sbs[nm]
                ts = slice(t * DL, (t + 1) * DL)
                dst = (qtiles if nm == "q" else ktiles)[m]
                ps = proj_ps.tile([P, DL], dt.float32, tag="proj")
                for kc in range(KC):
                    nc.tensor.matmul(
                        ps, lhsT=w_sb[:, kc, m * P:(m + 1) * P],
                        rhs=xT_sb[:, kc, ts],
                        start=(kc == 0), stop=(kc == KC - 1))
                raw = bstage.tile([P, DL], dt.bfloat16, tag="raw", bufs=2)
                nc.vector.tensor_scalar(raw, ps, b_sb[:, m:m + 1], None,
                                        mybir.AluOpType.add)
                # rotate-half in the permuted basis: swap the 32-row
                # halves of each head with SBUF->SBUF DMAs on the idle
                # sync/vector queues (sign lives in sinT)
                rsw = bstage.tile([P, DL], dt.bfloat16, tag="rsw", bufs=2)
                for hh in range(2):
                    o32 = hh * HEAD_DIM
                    ea, eb = (nc.sync, nc.sync) if hh == 0 else \
                        (nc.gpsimd, nc.gpsimd)
                    ea.dma_start(out=rsw[o32:o32 + HH, :],
                                 in_=raw[o32 + HH:o32 + HEAD_DIM, :])
                    eb.dma_start(out=rsw[o32 + HH:o32 + HEAD_DIM, :],
                                 in_=raw[o32:o32 + HH, :])
                t1 = bstage.tile([P, DL], dt.bfloat16, tag="t1", bufs=1, padded_shape=None)
                nc.vector.tensor_mul(t1, raw, cos_sb[:, ts])
                nc.vector.tensor_mul(dst[:, ts], rsw, sin_sb[:, ts])
                nc.vector.tensor_add(dst[:, ts], dst[:, ts], t1)

            def emit_v(kb):
                # v projection: natural [seq, dims] layout + bias matmul
                ps = proj_ps.tile([P, DL], dt.float32, tag="proj", name="ps_v")
                for kc in range(KC):
                    nc.tensor.matmul(
                        ps, lhsT=xT_sb[:, kc, kb * P:(kb + 1) * P],
                        rhs=w_sbs["v"][:, kc, :],
                        start=(kc == 0), stop=False)
                nc.tensor.matmul(
                    ps, lhsT=ones_sb, rhs=b_sbs["v"],
                    start=False, stop=True)
                nc.scalar.activation(
                    out=v_sb[:, kb, :, 0:HEAD_DIM],
                    in_=ps.rearrange("p (h d) -> p h d", h=HPC), func=FCopy)

            def gen_b_group(items):
                par = 0
                for it in items:
                    if it[0] == "v":
                        emit_v(it[1])
                    else:
                        nm, t, m = it
                        if m not in (qtiles if nm == "q" else ktiles):
                            pool = qpool if nm == "q" else kpool
                            tl = pool.tile([P, S], dt.bfloat16, tag=nm,
                                           name=f"{nm}{m}")
                            (qtiles if nm == "q" else ktiles)[m] = tl
                        emit_qk_tile(nm, t, m, par)
                        par ^= 1
                    yield

            # ---------------- stage C emitters ----------------
            strip_par = [0]

            def emit_c1_steps(h, att):
                # yields once per ck after emitting QK+exp(+mask)
                m, po = h // 2, (h % 2) * HEAD_DIM
                kh = ktiles[m][po:po + HEAD_DIM, :]
                qh = qtiles[m][po:po + HEAD_DIM, :]
                for ck in range(NKB):
                    w = S - ck * P
                    base = ck * P
                    off = ATT_OFF[ck]
                    for s0 in range(0, w, QKW):
                        sw = min(QKW, w - s0)
                        pool = qk_psA if strip_par[0] == 0 else qk_psB
                        strip_par[0] ^= 1
                        ps = pool.tile([P, QKW], dt.float32, tag="qk",
                                       name="ps_qk")
                        for u0 in range(0, sw, DL):
                            uw = min(DL, sw - u0)
                            nc.tensor.matmul(
                                ps[:, u0:u0 + uw],
                                lhsT=kh[:, ck * P:(ck + 1) * P],
                                rhs=qh[:, base + s0 + u0:base + s0 + u0 + uw],
                                start=True, stop=True)
                        nc.scalar.activation(
                            out=att[:, off + s0:off + s0 + sw],
                            in_=ps[:, 0:sw], func=FExp, scale=SCALE)
                    nc.gpsimd.tensor_mul(
                        att[:, off:off + P], att[:, off:off + P], tri_sb)
                    yield

            def emit_c2_steps(h, att):
                # baseline-orientation AV: att block is the (full-array)
                # stationary, v streams 65 cols; ones column of v makes
                # psum col 64 the softmax denominator; yields per qb
                m, po = h // 2, (h % 2) * HEAD_DIM
                for qb in range(NKB - 1, -1, -1):
                    ys = yT_ps_pool.tile([P, HEAD_DIM + 1], dt.float32,
                                         tag="yT", name="ys")
                    for ck in range(qb + 1):
                        a0 = ATT_OFF[ck] + (qb - ck) * P
                        nc.tensor.matmul(
                            ys, lhsT=att[:, a0:a0 + P],
                            rhs=v_sb[:, ck, h, :],
                            start=(ck == 0), stop=(ck == qb))
                    r = rrow.tile([P, 1], dt.float32, tag="rec")
                    nc.vector.reciprocal(r, ys[:, HEAD_DIM:HEAD_DIM + 1])
                    nc.vector.tensor_scalar(
                        get_ymb(m)[:, qb, po:po + HEAD_DIM],
                        ys[:, 0:HEAD_DIM], r, None,
                        mybir.AluOpType.mult)
                    yield

            def emit_ytrans(m, q4):
                tp = proj_ps.tile([P, 4 * P], dt.bfloat16, tag="proj",
                                  name="tp")
                for j in range(4):
                    nc.tensor.transpose(
                        tp[:, j * P:(j + 1) * P],
                        get_ymb(m)[:, q4 * 4 + j, :], ident_sb)
                nc.vector.tensor_copy(
                    out=yT_all[:, m, q4 * 4 * P:(q4 + 1) * 4 * P],
                    in_=tp)

            # ---------------- stage D emitter ----------------
            def emit_d(qb, t, par):
                pool = qk_psA if par else qk_psB
                ps = pool.tile([P, DL], dt.float32, tag="qk", name="ps_o")
                for m in range(MB):
                    nc.tensor.matmul(
                        ps, lhsT=yT_all[:, m, qb * P:(qb + 1) * P],
                        rhs=wo_sb[:, m, t * DL:(t + 1) * DL],
                        start=(m == 0), stop=(m == MB - 1))
                ob = osb.tile([P, DL], dt.bfloat16, tag="ob")
                if par:
                    nc.scalar.activation(out=ob, in_=ps, func=FCopy)
                else:
                    nc.vector.tensor_copy(out=ob, in_=ps)
                nc.sync.dma_start(
                    out=o_d[qb * P:(qb + 1) * P, t * DL:(t + 1) * DL],
                    in_=ob)

            # ---------------- weave ----------------
            def weave(*gens):
                alive = list(gens)
                while alive:
                    alive = [g for g in alive if next(g, "done") != "done"]

            def chain(*gens):
                for g in gens:
                    yield from g

            def ratio(g, n):
                while True:
                    for _ in range(n):
                        if next(g, "done") == "done":
                            return
                    yield

            with (
                tc.tile_pool(name="bweights", bufs=1) as bweights,
                tc.tile_pool(name="bstage", bufs=2) as bstage,
            ):
                xT_sb = bweights.tile([P, KC, S], dt.bfloat16, tag="xT")
                w_sbs = {nm: bweights.tile([P, KC, DL], dt.bfloat16,
                                           tag=f"w{nm}", name=f"w{nm}")
                         for nm in ("q", "k", "v")}
                cos_sb = bweights.tile([P, S], dt.bfloat16, tag="cos")
                sin_sb = bweights.tile([P, S], dt.bfloat16, tag="sin")

                # input DMAs.  sync: biases + x; scalar: q/k weights +
                # tables (idle until the first exp); vector: v weights.
                nc.sync.dma_start(out=b_sbs["q"], in_=bq_d)
                nc.sync.dma_start(out=b_sbs["k"], in_=bk_d)
                for kc in range(KC):
                    nc.scalar.dma_start(out=w_sbs["q"][:, kc, :],
                                        in_=wq_d[kc * P:(kc + 1) * P, :])
                for t in range(NT):
                    eng = nc.sync if t < 2 else nc.gpsimd
                    for kc in range(KC):
                        eng.dma_start(
                            out=xT_sb[:, kc, t * DL:(t + 1) * DL],
                            in_=xT_d[kc * P:(kc + 1) * P, t * DL:(t + 1) * DL])
                nc.scalar.dma_start(out=cos_sb, in_=cos_d)
                nc.scalar.dma_start(out=sin_sb, in_=sin_d)
                for kc in range(KC):
                    nc.scalar.dma_start(out=w_sbs["k"][:, kc, :],
                                        in_=wk_d[kc * P:(kc + 1) * P, :])
                nc.scalar.dma_start(out=tri_sb, in_=tri_d)
                nc.scalar.dma_start(out=ident_sb, in_=ident_d)
                for kc in range(KC):
                    nc.gpsimd.dma_start(out=w_sbs["v"][:, kc, :],
                                        in_=wv_d[kc * P:(kc + 1) * P, :])
                nc.gpsimd.dma_start(out=b_sbs["v"], in_=bv_d)

                bm = [gen_b_group([(nm, t, m) for nm in ("q", "k")
                                   for t in range(NT)])
                      for m in range(MB)]
                bv_gen = gen_b_group([("v", kb) for kb in range(NKB)])

                weave(bm[0])
                weave(bv_gen, emit_c1_steps(0, get_att(0)))
                weave(bm[1], ratio(emit_c1_steps(1, get_att(1)), 2))
                weave(bm[2], chain(emit_c2_steps(0, get_att(0)),
                                   ratio(emit_c1_steps(2, get_att(2)), 2)))
                # c2 must catch up to one behind c1 before the steady
                # pipeline (att slots cycle with period 2)
                weave(bm[3], chain(emit_c2_steps(1, get_att(1)),
                                   emit_c2_steps(2, get_att(2)),
                                   ratio(emit_c1_steps(3, get_att(3)), 2)))

            # B pools closed: allocate stage-D tiles in the freed space
            with (
                tc.tile_pool(name="wop", bufs=1) as wop,
                tc.tile_pool(name="osb", bufs=2) as osb,
            ):
                wo_sb = wop.tile([P, MB, D_MODEL], dt.bfloat16, tag="wo")
                nc.gpsimd.dma_start(
                    out=wo_sb, in_=wo_d.rearrange("(m p) n -> p m n", p=P))

                # c2(0), c2(1) complete -> pair m=0 transposable
                for q4 in range(4):
                    emit_ytrans(0, q4)
                for h in range(4, HPC):
                    weave(ratio(emit_c1_steps(h, get_att(h)), 4),
                          emit_c2_steps(h - 1, get_att(h - 1)))
                    if h == 4:
                        for q4 in range(4):
                            emit_ytrans(1, q4)   # after c2(3)
                    if h == 6:
                        for q4 in range(4):
                            emit_ytrans(2, q4)   # after c2(5)
                # last head's AV woven with m=3 transposes + out-projection
                c2_last = emit_c2_steps(HPC - 1, get_att(HPC - 1))
                par = 0
                for q4 in range(3, -1, -1):
                    for _ in range(4):
                        next(c2_last, None)
                    emit_ytrans(3, q4)
                    for qb in range(4 * q4 + 3, 4 * q4 - 1, -1):
                        for t in range(2):
                            emit_d(qb, t, par)
                            par ^= 1

    nc.compile()
    return nc


def _perm64():
    # pi: permuted-basis index j -> original head dim (evens then odds)
    return np.concatenate([np.arange(0, HEAD_DIM, 2), np.arange(1, HEAD_DIM, 2)])


def _host_tables():
    pos = np.arange(S, dtype=np.float32)
    freq = np.arange(0, HEAD_DIM, 2, dtype=np.float32) / HEAD_DIM
    inv_freq = 1.0 / (10000.0 ** freq)                  # [32]
    ang = np.outer(inv_freq, pos)                       # [32, S]
    cos1 = np.cos(ang)
    sin1 = np.sin(ang)
    # pi-basis per-head tables [64, S]: rows 0..31 evens, 32..63 odds
    cosh = np.concatenate([cos1, cos1], axis=0)
    sinh = np.concatenate([-sin1, sin1], axis=0)        # sign folded in
    cosT = np.tile(cosh, (2, 1))                        # [128, S] head pair
    sinT = np.tile(sinh, (2, 1))
    tri = np.triu(np.ones((P, P), np.float32))          # keep k<=q in [k,q]
    ident = np.eye(P, dtype=np.float32)
    return (cosT.astype(BF16), sinT.astype(BF16), tri.astype(BF16),
            ident.astype(BF16))


def kernel(x, Wq, bq, Wk, bk, Wv, bv, Wo, bo):
    from concourse.bass_utils import run_bass_kernel_spmd

    x = np.asarray(x, np.float32)
    Wq, Wk, Wv, Wo = (np.asarray(a, np.float32) for a in (Wq, Wk, Wv, Wo))
    bq, bk, bv, bo = (np.asarray(a, np.float32) for a in (bq, bk, bv, bo))

    if "nc" not in _CACHE:
        _CACHE["nc"] = _build_bass()
    nc = _CACHE["nc"]

    cosT, sinT, tri, ident = _host_tables()
    consts = {"cosT": cosT, "sinT": sinT, "tri": tri, "ident": ident}

    # pi-basis permutation of q/k projection columns (per head)
    pi = _perm64()
    colperm = np.concatenate([h * HEAD_DIM + pi for h in range(N_HEADS)])
    Wq_p = Wq[:, colperm]
    Wk_p = Wk[:, colperm]
    bq_p = bq[colperm]
    bk_p = bk[colperm]

    xTs = [np.ascontiguousarray(x[b].T).astype(BF16) for b in range(B)]
    in_maps = []
    for c in range(N_CORES):
        b, g = c // HG, c % HG
        sl = slice(g * DL, (g + 1) * DL)
        in_maps.append({
            "xT": xTs[b],
            "wq": np.ascontiguousarray(Wq_p[:, sl]).astype(BF16),
            "wk": np.ascontiguousarray(Wk_p[:, sl]).astype(BF16),
            "wv": np.ascontiguousarray(Wv[:, sl]).astype(BF16),
            "wo": np.ascontiguousarray(Wo[sl, :]).astype(BF16),
            "bqT": np.ascontiguousarray(
                bq_p[sl].reshape(MB, P).T).astype(np.float32),
            "bkT": np.ascontiguousarray(
                bk_p[sl].reshape(MB, P).T).astype(np.float32),
            "bv": bv[sl].reshape(1, DL).astype(BF16),
            **consts,
        })

    res = run_bass_kernel_spmd(nc, in_maps, core_ids=list(range(N_CORES)))
    _CACHE["last_result"] = res
    out = np.empty((B, S, D_MODEL), np.float32)
    for b in range(B):
        out[b] = (res.results[HG * b]["o"].astype(np.float32) +
                  res.results[HG * b + 1]["o"].astype(np.float32))
    out += bo.astype(np.float32)
    return out
